# revision 13
# baseline (speedup 1.0000x reference)
"""Trainium2 Bass kernel for the differentiable Gaussian renderer.

Math: for each pose, each gaussian g splats w[g,p] = op_g * exp(-0.5*d2/var_g)
onto pixels p; output = (sum_g w*color) / (sum_g w + n_chunks*eps), tiled.

Key structure exploited: the Gaussian is separable, exp(-(dx^2+dy^2)*s) =
Ex(c) * Ey(r), where dx depends only on the pixel column and dy only on the
row.  Per gaussian we need just 256 exp evaluations instead of 16384, and the
pixel accumulation becomes, per 128-gaussian chunk, one K=128 matmul:

  acc[r, ch*128+c] += sum_g Ey[g,r] * rhs[g, ch*128+c],
  rhs[g, ch*128+c] = Ex[g,c] * colors[g, ch]   (ch==3 column is Ex itself,
                                                giving the denominator)

The exp arguments arg_x[g,c] = niv_g*(c'-u'_g)^2 + ln(op_g) (and arg_y) are
produced on the tensor engine: per 4-chunk block, the per-gaussian bf16
coefficients (each split 3-way hi/mid/lo for ~1e-4 absolute accuracy, 24
q-slots per chunk) are PE-transposed into a [24*4, 128] layout, then two
K=96 bf16 matmuls against a block-diagonal constant matrix of {1, c', c'^2}
rows produce the block's x-args and y-args (one PSUM bank each).  Opacity
rides in ln-space inside arg_x; u,v are clamped to +-110.5 around the image
center (gaussians beyond that have w == 0 in fp32 anyway).

Sharding: gaussians are split 8 ways (8192/core).  Each core accumulates
partial num/den for a pose in a single PSUM bank (64 chunk matmuls), then a
ReduceScatter(add) combines partials across cores: core c receives image
rows 16c..16c+16, divides, and writes only its shard; the host gathers the
8 shards into the full image.  Pose 0's ReduceScatter overlaps pose 1's
rendering; a tiny warm-up AllReduce at program start absorbs the ~30us CC
bootstrap barrier so the first real collective launches immediately.

Pipeline: per 4-chunk block, phase A = PE transpose -> 2 arg-MMs -> ACT exp
(bf16 out) -> color scale (broadcast multiply, alternating DVE / GpSimd);
the block's 4 bf16 main-MMs are emitted one block behind so the PE always
has phase-A work while colors finish.
"""
import os
import numpy as np
import ml_dtypes

import concourse.mybir as mybir
import concourse.tile as tile
import concourse.bacc as bacc
from concourse.bass_utils import run_bass_kernel_spmd

f32 = mybir.dt.float32
bf16 = mybir.dt.bfloat16
ALU = mybir.AluOpType
ACTF = mybir.ActivationFunctionType

NCORES = 8
NPOSE = 2
H = W = 128
FX = FY = 120.0
CX = CY = 64.0
NG = 65536
NGC = NG // NCORES          # gaussians per core
NCHUNK = NGC // 128         # 64 chunks of 128 gaussians
NBLK = NCHUNK // 4          # 16 transpose blocks of 4 chunks
CENT = 63.5
UCLAMP = 110.5
NQ = 24                     # q-slots per chunk
RSROWS = H // NCORES        # image rows per core after ReduceScatter


def _bf(x):
    return np.asarray(x).astype(ml_dtypes.bfloat16)


def _split3(x):
    h = _bf(x).astype(np.float64)
    m = _bf(x - h).astype(np.float64)
    l = _bf(x - h - m).astype(np.float64)
    return h, m, l


def _const_blocks():
    """[96, 1024] bf16 block-diagonal matmul constant.  Block jj (rows
    24jj.., x-cols 128jj.., y-cols 512+128jj..) holds the per-q constant
    rows for chunk jj of a 4-chunk transpose block.

    q-slot layout per chunk (matches build_packed writes):
      0-3   a_x, b_x, a_y, b_y  (hi)
      4-7   same (mid)
      8-11  same (lo)
      12-17 x quadratic: nivh*c2h, nivh*c2m, nivh*c2l, nivm*c2h,
            nivm*c2m, nivl*c2h
      18-23 y quadratic, mirrored
    """
    cp = np.arange(128, dtype=np.float64) - CENT
    c2 = cp * cp
    c2h, c2m, c2l = _split3(c2)
    ones = np.ones(128)
    zer = np.zeros(128)
    xrows, yrows = [], []
    for _ in range(3):                       # h / m / l coefficient groups
        xrows += [ones, cp, zer, zer]
        yrows += [zer, zer, ones, cp]
    xrows += [c2h, c2m, c2l, c2h, c2m, c2h] + [zer] * 6
    yrows += [zer] * 6 + [c2h, c2m, c2l, c2h, c2m, c2h]
    bx, by = np.stack(xrows), np.stack(yrows)
    # packed stores a 4-chunk block as [q, a] interleaved rows (a = chunk
    # within block), so const row q*4+a carries chunk a's columns only
    cxy = np.zeros((96, 1024))
    for q in range(NQ):
        for a in range(4):
            cxy[q * 4 + a, 128 * a:128 * a + 128] = bx[q]
            cxy[q * 4 + a, 512 + 128 * a:512 + 128 * a + 128] = by[q]
    return _bf(cxy)


def _quat2mat(q):
    q = np.asarray(q, np.float64)
    q = q / np.linalg.norm(q)
    w, x, y, z = q
    return np.array([
        [1 - 2 * (y * y + z * z), 2 * (x * y - z * w), 2 * (x * z + y * w)],
        [2 * (x * y + z * w), 1 - 2 * (x * x + z * z), 2 * (y * z - x * w)],
        [2 * (x * z - y * w), 2 * (y * z + x * w), 1 - 2 * (x * x + y * y)],
    ])


def _build(eps_total: float):
    nc = bacc.Bacc("TRN2", target_bir_lowering=False, debug=False,
                   num_devices=NCORES)
    # host pre-laid-out inputs: partition p holds gaussian j*128+p at free j
    pos = nc.dram_tensor("positions", [128, NCHUNK, 3], f32, kind="ExternalInput")
    col = nc.dram_tensor("colors", [128, NCHUNK, 3], f32, kind="ExternalInput")
    opa = nc.dram_tensor("opacities", [128, NCHUNK], f32, kind="ExternalInput")
    sca = nc.dram_tensor("scales", [128, NCHUNK], f32, kind="ExternalInput")
    rt = nc.dram_tensor("rt", [NPOSE, 12], f32, kind="ExternalInput")
    out = nc.dram_tensor("out", [NPOSE, RSROWS, 3, W], f32,
                         kind="ExternalOutput")

    cxy = _const_blocks()
    constxy_d = nc.inline_tensor(np.asarray(cxy), name="constXY")
    ident_d = nc.inline_tensor(np.eye(128, dtype=ml_dtypes.bfloat16), name="ident")
    warmz_d = nc.inline_tensor(np.zeros(128, np.float32), name="warmz")

    cc_in = nc.dram_tensor("cc_in", [NPOSE, 128, 512], f32, kind="Internal")
    cc_out = nc.dram_tensor("cc_out", [NPOSE, RSROWS, 512], f32,
                            kind="Internal")
    warm_out = nc.dram_tensor("warm_out", [128], f32, kind="Internal",
                              addr_space="Shared")

    groups = [list(range(NCORES))]

    with tile.TileContext(nc) as tc:
        with (
            tc.tile_pool(name="const", bufs=1) as cpool,
            tc.tile_pool(name="prep", bufs=2) as prep,
            tc.tile_pool(name="pk", bufs=2) as pkpool,
            tc.tile_pool(name="t32", bufs=3) as t32pool,
            tc.tile_pool(name="rhs4", bufs=3) as rhspool,
            tc.tile_pool(name="ey4", bufs=3) as eypool,
            tc.tile_pool(name="fin", bufs=1) as fin,
            tc.tile_pool(name="ps_tr", bufs=2, space="PSUM") as ps_tr,
            tc.tile_pool(name="ps_arg", bufs=2, space="PSUM") as ps_arg,
            tc.tile_pool(name="ps_acc", bufs=2, space="PSUM") as ps_acc,
        ):
            # ---- warm-up collective: absorbs the CC bootstrap barrier ----
            nc.gpsimd.collective_compute(
                "AllReduce", ALU.add, replica_groups=groups,
                ins=[warmz_d.ap().opt()], outs=[warm_out.ap().opt()])

            # ---- constants / inputs to SBUF (contiguous DMAs) ----
            constxy = cpool.tile([96, 1024], bf16)
            nc.sync.dma_start(constxy[:], constxy_d.ap())
            ident = cpool.tile([128, 128], bf16)
            nc.sync.dma_start(ident[:], ident_d.ap())
            pos_t = cpool.tile([128, NCHUNK, 3], f32)
            nc.sync.dma_start(pos_t[:], pos.ap())
            col3 = cpool.tile([128, NCHUNK, 3], f32)
            nc.sync.dma_start(col3[:], col.ap())
            opat = cpool.tile([128, NCHUNK], f32)
            nc.sync.dma_start(opat[:], opa.ap())
            scat = cpool.tile([128, NCHUNK], f32)
            nc.sync.dma_start(scat[:], sca.ap())
            rtb = cpool.tile([128, NPOSE * 12], f32)
            nc.sync.dma_start(rtb[:], rt.ap().rearrange("a b -> (a b)")
                              .partition_broadcast(128))

            # ---- pose-independent per-gaussian prep ----
            opc = prep.tile([128, NCHUNK], f32, tag="opc")
            nc.vector.tensor_scalar_max(opc[:], opat[:], 1e-30)
            lnop = prep.tile([128, NCHUNK], f32, tag="lnop")
            nc.scalar.activation(lnop[:], opc[:], ACTF.Ln)

            s2 = prep.tile([128, NCHUNK], f32, tag="s2")
            nc.vector.tensor_tensor(s2[:], scat[:], scat[:], ALU.mult)
            m2s2 = prep.tile([128, NCHUNK], f32, tag="m2s2")
            nc.vector.tensor_scalar_mul(m2s2[:], s2[:], -2.0)
            niv = prep.tile([128, NCHUNK], f32, tag="niv")
            nc.vector.reciprocal(niv[:], m2s2[:])

            nivh = prep.tile([128, NCHUNK], bf16, tag="nivh")
            nc.vector.tensor_copy(nivh[:], niv[:])
            r1 = prep.tile([128, NCHUNK], f32, tag="r1")
            nc.vector.tensor_tensor(r1[:], niv[:], nivh[:], ALU.subtract)
            nivm = prep.tile([128, NCHUNK], bf16, tag="nivm")
            nc.vector.tensor_copy(nivm[:], r1[:])
            r2 = prep.tile([128, NCHUNK], f32, tag="r2")
            nc.vector.tensor_tensor(r2[:], r1[:], nivm[:], ALU.subtract)
            nivl = prep.tile([128, NCHUNK], bf16, tag="nivl")
            nc.vector.tensor_copy(nivl[:], r2[:])

            def rsc(p, k):
                return rtb[:, p * 12 + k: p * 12 + k + 1]

            def nivv(t):
                return (t[:].rearrange("p (b a) -> p b a", b=NBLK)
                        .unsqueeze(2))

            def build_packed(p):
                """Per-pose packed coefficient tile, block-major
                (bf16, [128, NBLK, NQ, 4]): slice [:, bb] is a contiguous
                [128, 96] transpose input."""
                packed = pkpool.tile([128, NBLK, NQ, 4], bf16, tag="pk",
                                     name=f"packed{p}")
                # quadratic niv slots (broadcast copies)
                nc.vector.tensor_copy(
                    packed[:, :, 12:15, :],
                    nivv(nivh).broadcast_to([128, NBLK, 3, 4]))
                nc.vector.tensor_copy(
                    packed[:, :, 15:17, :],
                    nivv(nivm).broadcast_to([128, NBLK, 2, 4]))
                nc.vector.tensor_copy(
                    packed[:, :, 17:18, :],
                    nivv(nivl).broadcast_to([128, NBLK, 1, 4]))
                nc.vector.tensor_copy(
                    packed[:, :, 18:21, :],
                    nivv(nivh).broadcast_to([128, NBLK, 3, 4]))
                nc.vector.tensor_copy(
                    packed[:, :, 21:23, :],
                    nivv(nivm).broadcast_to([128, NBLK, 2, 4]))
                nc.vector.tensor_copy(
                    packed[:, :, 23:24, :],
                    nivv(nivl).broadcast_to([128, NBLK, 1, 4]))

                cam = []
                for crow in range(3):
                    acc = prep.tile([128, NCHUNK], f32, tag=f"cam{crow}")
                    nc.vector.tensor_scalar(acc[:], pos_t[:, :, 0],
                                            rsc(p, 3 * crow + 0),
                                            rsc(p, 9 + crow), ALU.mult, ALU.add)
                    for k in (1, 2):
                        t_ = prep.tile([128, NCHUNK], f32, tag="cam_t")
                        nc.vector.tensor_scalar_mul(t_[:], pos_t[:, :, k],
                                                    rsc(p, 3 * crow + k))
                        nc.vector.tensor_tensor(acc[:], acc[:], t_[:], ALU.add)
                    cam.append(acc)

                zr = prep.tile([128, NCHUNK], f32, tag="zr")
                nc.vector.reciprocal(zr[:], cam[2][:])

                # cf layout: [a_x, b_x, a_y, b_y] matching q-slots 0..3
                cf = prep.tile([128, 4, NCHUNK], f32, tag="cf")
                for ax_i, (ci, foc, off) in enumerate(
                        ((cam[0], FX, CX), (cam[1], FY, CY))):
                    t_ = prep.tile([128, NCHUNK], f32, tag="uv_t")
                    nc.vector.tensor_tensor(t_[:], ci[:], zr[:], ALU.mult)
                    u_ = prep.tile([128, NCHUNK], f32, tag="uv_u")
                    nc.vector.tensor_scalar(u_[:], t_[:], float(foc),
                                            float(off - CENT), ALU.mult, ALU.add)
                    ucl = prep.tile([128, NCHUNK], f32, tag="uv_ucl")
                    nc.vector.tensor_scalar(ucl[:], u_[:], -UCLAMP, UCLAMP,
                                            ALU.max, ALU.min)
                    w2 = prep.tile([128, NCHUNK], f32, tag="w2")
                    nc.vector.tensor_tensor(w2[:], ucl[:], ucl[:], ALU.mult)
                    a_v = cf[:, 2 * ax_i, :]
                    nc.vector.tensor_tensor(a_v, w2[:], niv[:], ALU.mult)
                    if ax_i == 0:
                        nc.vector.tensor_tensor(a_v, a_v, lnop[:], ALU.add)
                    b_f = prep.tile([128, NCHUNK], f32, tag="b_f")
                    nc.vector.tensor_tensor(b_f[:], ucl[:], niv[:], ALU.mult)
                    nc.vector.tensor_scalar_mul(cf[:, 1 + 2 * ax_i, :],
                                                b_f[:], -2.0)

                cfv = cf[:].rearrange("p c (b a) -> p b c a", b=NBLK)
                nc.vector.tensor_copy(packed[:, :, 0:4, :], cfv)
                sr1 = prep.tile([128, NBLK, 4, 4], f32, tag="sr1")
                nc.vector.tensor_tensor(sr1[:], cfv, packed[:, :, 0:4, :],
                                        ALU.subtract)
                nc.vector.tensor_copy(packed[:, :, 4:8, :], sr1[:])
                sr2 = prep.tile([128, NBLK, 4, 4], f32, tag="sr2")
                nc.vector.tensor_tensor(sr2[:], sr1[:], packed[:, :, 4:8, :],
                                        ALU.subtract)
                nc.vector.tensor_copy(packed[:, :, 8:12, :], sr2[:])
                return packed

            def phase_a(p, packed, bb):
                """Transpose + arg MMs + exp + colors for 4-chunk block bb.
                Returns (ey4, rhs4) bf16 tiles."""
                ptr = ps_tr.tile([128, 128], bf16, tag="tr")
                nc.tensor.transpose(
                    ptr[0:96, :],
                    packed[:, bb].rearrange("p q a -> p (q a)"),
                    ident[:])
                t32 = t32pool.tile([128, 128], bf16, tag="t32")
                nc.vector.tensor_copy(t32[0:96, :], ptr[0:96, :])

                parg = ps_arg.tile([128, 1024], f32, tag="arg")
                nc.tensor.matmul(parg[:, 0:512], t32[0:96, :],
                                 constxy[:, 0:512], start=True, stop=True)
                nc.tensor.matmul(parg[:, 512:1024], t32[0:96, :],
                                 constxy[:, 512:1024], start=True, stop=True)
                rhs4 = rhspool.tile([128, 4, 512], bf16, tag="rhs4")
                ey4 = eypool.tile([128, 4, 128], bf16, tag="ey4")
                nc.scalar.activation(rhs4[:, :, 384:512],
                                     parg[:, 0:512]
                                     .rearrange("p (a x) -> p a x", a=4),
                                     ACTF.Exp)
                nc.scalar.activation(ey4[:, :, :],
                                     parg[:, 512:1024]
                                     .rearrange("p (a x) -> p a x", a=4),
                                     ACTF.Exp)
                eng = nc.vector if bb % 2 == 0 else nc.gpsimd
                eng.tensor_tensor(
                    rhs4[:, :, 0:384].rearrange("p a (c x) -> p a c x", c=3),
                    rhs4[:, :, 384:512].unsqueeze(2)
                    .broadcast_to([128, 4, 3, 128]),
                    col3[:, 4 * bb: 4 * bb + 4, :].unsqueeze(3)
                    .broadcast_to([128, 4, 3, 128]),
                    ALU.mult)
                return ey4, rhs4

            packed_all = [None, None]
            shards = []
            for p in range(NPOSE):
                if p == 0:
                    packed_all[0] = build_packed(0)
                packed = packed_all[p]
                pacc = ps_acc.tile([128, 512], f32, tag="acc")
                pending = None
                for bb in range(NBLK):
                    tiles = phase_a(p, packed, bb)
                    if p == 0 and bb == 0:
                        # pose-1 prep fills the DVE while the PE chews on
                        # pose-0 blocks
                        packed_all[1] = build_packed(1)
                    if pending is not None:
                        ey4, rhs4 = pending
                        for k in range(4):
                            nc.tensor.matmul(
                                pacc[:], ey4[:, k, :], rhs4[:, k, :],
                                start=(bb == 1 and k == 0), stop=False)
                    pending = tiles
                ey4, rhs4 = pending
                for k in range(4):
                    nc.tensor.matmul(pacc[:], ey4[:, k, :], rhs4[:, k, :],
                                     start=False, stop=(k == 3))

                acc_sb = fin.tile([128, 512], f32, tag=f"accsb{p}")
                nc.scalar.copy(acc_sb[:], pacc[:])
                nc.sync.dma_start(cc_in.ap()[p], acc_sb[:])
                nc.gpsimd.collective_compute(
                    "ReduceScatter", ALU.add, replica_groups=groups,
                    ins=[cc_in.ap()[p].opt()],
                    outs=[cc_out.ap()[p].opt()])
                shard = fin.tile([RSROWS, 512], f32, tag=f"shard{p}")
                nc.sync.dma_start(shard[:], cc_out.ap()[p])
                shards.append(shard)

            # ---- final: divide own shard, write it out ----
            for p in range(NPOSE):
                shard = shards[p]
                dplus = fin.tile([RSROWS, 128], f32, tag=f"dplus{p}")
                nc.vector.tensor_scalar_add(dplus[:], shard[:, 384:512],
                                            float(eps_total))
                rcp = fin.tile([RSROWS, 128], f32, tag=f"rcp{p}")
                nc.vector.reciprocal(rcp[:], dplus[:])
                img = fin.tile([RSROWS, 3, 128], f32, tag=f"img{p}")
                nc.vector.tensor_tensor(
                    img[:], shard[:, 0:384].rearrange("p (c x) -> p c x", c=3),
                    rcp[:].unsqueeze(1).broadcast_to([RSROWS, 3, 128]),
                    ALU.mult)
                nc.sync.dma_start(out.ap()[p], img[:])

    nc.compile()
    return nc


def _install_ntff_hook():
    """Best-effort: register the axon NTFF profiling hook so trace=True
    yields exec_time_ns + a perfetto trace. The image's antenv package
    lacks axon_hooks, so bass_utils would otherwise skip tracing."""
    try:
        import sys
        import types
        mod = sys.modules.get("antenv.axon_hooks")
        if mod is None:
            import antenv
            mod = types.ModuleType("antenv.axon_hooks")
            mod._hook = None
            mod.set_axon_ntff_profile_hook = lambda h: setattr(mod, "_hook", h)
            mod.get_axon_ntff_profile_hook = lambda: mod._hook
            sys.modules["antenv.axon_hooks"] = mod
            antenv.axon_hooks = mod
        if mod.get_axon_ntff_profile_hook() is None:
            from trn_agent_boot.trn_boot import _ntff_profile_via_ctypes
            hook = _ntff_profile_via_ctypes("/opt/axon/libaxon_pjrt.so")
            if hook is None:
                return False
            mod.set_axon_ntff_profile_hook(hook)
        return True
    except Exception:
        return False


_CACHE = {}


def _get_nc(eps_total: float):
    key = float(eps_total)
    if key not in _CACHE:
        _CACHE[key] = _build(key)
    return _CACHE[key]


def kernel(positions, colors, opacities, scales, qvec, tvec,
           tile_hw=32, chunk_gauss=4096):
    positions = np.asarray(positions, np.float32)
    colors = np.asarray(colors, np.float32)
    opacities = np.asarray(opacities, np.float32)
    scales = np.asarray(scales, np.float32)
    qvec = np.asarray(qvec, np.float32)
    tvec = np.asarray(tvec, np.float32)
    tile_hw = int(tile_hw)
    chunk_gauss = int(chunk_gauss)
    n = positions.shape[0]
    assert n == NG and tile_hw == 32, (n, tile_hw)
    eps_total = (n // chunk_gauss) * 1e-8

    rtv = np.zeros((NPOSE, 12), np.float32)
    for p in range(NPOSE):
        rtv[p, :9] = _quat2mat(qvec[p]).astype(np.float32).reshape(9)
        rtv[p, 9:12] = tvec[p]

    def lay(a, shape):
        return np.ascontiguousarray(
            a.reshape(NCHUNK, 128, -1).transpose(1, 0, 2).reshape(shape))

    in_maps = []
    for c in range(NCORES):
        sl = slice(c * NGC, (c + 1) * NGC)
        in_maps.append({
            "positions": lay(positions[sl], (128, NCHUNK, 3)),
            "colors": lay(colors[sl], (128, NCHUNK, 3)),
            "opacities": lay(opacities[sl], (128, NCHUNK)),
            "scales": lay(scales[sl], (128, NCHUNK)),
            "rt": rtv,
        })

    nc = _get_nc(eps_total)
    res = None
    trace = os.environ.get("KERNEL_TRACE", "1") == "1"
    if trace:
        trace = _install_ntff_hook()
    for attempt in range(3):
        try:
            res = run_bass_kernel_spmd(nc, in_maps, core_ids=list(range(NCORES)),
                                       trace=trace)
            break
        except Exception:
            if attempt == 2:
                raise
    global LAST_RESULT
    LAST_RESULT = res
    if res.exec_time_ns is not None:
        print(f"HW exec time: {res.exec_time_ns} ns")
    full = np.empty((NPOSE, 3, H, W), np.float32)
    for c in range(NCORES):
        sh = res.results[c]["out"]       # [NPOSE, RSROWS, 3, W]
        full[:, :, RSROWS * c:RSROWS * (c + 1), :] = sh.transpose(0, 2, 1, 3)
    return (full.reshape(NPOSE, 3, 16, 1024).transpose(0, 2, 1, 3)
            .reshape(NPOSE * 16, 3, tile_hw, tile_hw).astype(np.float32))


# revision 14
# speedup vs baseline: 1.0419x; 1.0419x over previous
"""Trainium2 Bass kernel for the differentiable Gaussian renderer.

Math: for each pose, each gaussian g splats w[g,p] = op_g * exp(-0.5*d2/var_g)
onto pixels p; output = (sum_g w*color) / (sum_g w + n_chunks*eps), tiled.

Key structure exploited: the Gaussian is separable, exp(-(dx^2+dy^2)*s) =
Ex(c) * Ey(r), where dx depends only on the pixel column and dy only on the
row.  Per gaussian we need just 256 exp evaluations instead of 16384, and the
pixel accumulation becomes, per 128-gaussian chunk, one K=128 matmul:

  acc[r, ch*128+c] += sum_g Ey[g,r] * rhs[g, ch*128+c],
  rhs[g, ch*128+c] = Ex[g,c] * colors[g, ch]   (ch==3 column is Ex itself,
                                                giving the denominator)

The exp arguments arg_x[g,c] = niv_g*(c'-u'_g)^2 + ln(op_g) (and arg_y) are
produced on the tensor engine: per 4-chunk block, the per-gaussian bf16
coefficients (split 3-way hi/mid/lo for ~1e-4 absolute accuracy, 24 q-slots
per chunk, block-major layout) are PE-transposed into a [4*24, 128] layout,
then two K=96 bf16 matmuls against a block-diagonal constant matrix of
{1, c', c'^2} rows produce the block's x-args and y-args (one PSUM bank
each).  Opacity rides in ln-space inside arg_x; u,v are clamped to +-110.5
around the image center (gaussians beyond that have w == 0 in fp32 anyway).

Sharding: gaussians are split 8 ways (8192/core).  Each core accumulates
partial num/den for both poses (one PSUM bank per pose, 64 chunk matmuls),
then a single ReduceScatter(add) over the concatenated [2,128,512] partials
combines them across cores: core c receives a contiguous 32-row shard
(cores 0-3 carry pose 0, cores 4-7 pose 1), divides, and writes only its
shard; the host gathers the 8 shards into the two full images.  One
collective per run keeps the ~35us CC bootstrap barrier (which cannot start
before ~22us regardless) off the critical path until rendering ends, and
the dependency chain pins the tiny divide at the true end of the program so
the tile scheduler cannot hoist it into the render stream.

Pipeline: per 4-chunk block, phase A = PE transpose -> 2 arg-MMs -> ACT exp
(bf16 out) -> color scale (broadcast bf16 multiply, 2 of 3 blocks on DVE,
1 of 3 on GpSimd); the block's 4 bf16 main-MMs are emitted one block behind
so the PE always has phase-A work while colors finish.
"""
import os
import numpy as np
import ml_dtypes

import concourse.mybir as mybir
import concourse.tile as tile
import concourse.bacc as bacc
from concourse.bass_utils import run_bass_kernel_spmd

f32 = mybir.dt.float32
bf16 = mybir.dt.bfloat16
ALU = mybir.AluOpType
ACTF = mybir.ActivationFunctionType

NCORES = 8
NPOSE = 2
H = W = 128
FX = FY = 120.0
CX = CY = 64.0
NG = 65536
NGC = NG // NCORES          # gaussians per core
NCHUNK = NGC // 128         # 64 chunks of 128 gaussians
NBLK = NCHUNK // 4          # 16 transpose blocks of 4 chunks
CENT = 63.5
UCLAMP = 110.5
NQ = 24                     # q-slots per chunk
RSROWS = NPOSE * H // NCORES   # rows per core after the ReduceScatter


def _bf(x):
    return np.asarray(x).astype(ml_dtypes.bfloat16)


def _split3(x):
    h = _bf(x).astype(np.float64)
    m = _bf(x - h).astype(np.float64)
    l = _bf(x - h - m).astype(np.float64)
    return h, m, l


def _const_blocks():
    """[96, 1024] bf16 block-diagonal matmul constant.  Row q*4+a carries
    chunk a's columns only (packed stores a 4-chunk block as [q, a]
    interleaved rows).

    q-slot layout per chunk (matches build_packed writes):
      0-3   a_x, a_y, b_x, b_y  (hi)
      4-7   same (mid)
      8-11  same (lo)
      12-17 x quadratic: nivh*c2h, nivh*c2m, nivh*c2l, nivm*c2h,
            nivm*c2m, nivl*c2h
      18-23 y quadratic, mirrored
    """
    cp = np.arange(128, dtype=np.float64) - CENT
    c2 = cp * cp
    c2h, c2m, c2l = _split3(c2)
    ones = np.ones(128)
    zer = np.zeros(128)
    xrows, yrows = [], []
    for _ in range(3):                       # h / m / l coefficient groups
        xrows += [ones, zer, cp, zer]
        yrows += [zer, ones, zer, cp]
    xrows += [c2h, c2m, c2l, c2h, c2m, c2h] + [zer] * 6
    yrows += [zer] * 6 + [c2h, c2m, c2l, c2h, c2m, c2h]
    bx, by = np.stack(xrows), np.stack(yrows)
    cxy = np.zeros((96, 1024))
    for q in range(NQ):
        for a in range(4):
            cxy[q * 4 + a, 128 * a:128 * a + 128] = bx[q]
            cxy[q * 4 + a, 512 + 128 * a:512 + 128 * a + 128] = by[q]
    return _bf(cxy)


def _quat2mat(q):
    q = np.asarray(q, np.float64)
    q = q / np.linalg.norm(q)
    w, x, y, z = q
    return np.array([
        [1 - 2 * (y * y + z * z), 2 * (x * y - z * w), 2 * (x * z + y * w)],
        [2 * (x * y + z * w), 1 - 2 * (x * x + z * z), 2 * (y * z - x * w)],
        [2 * (x * z - y * w), 2 * (y * z + x * w), 1 - 2 * (x * x + y * y)],
    ])


def _build(eps_total: float):
    nc = bacc.Bacc("TRN2", target_bir_lowering=False, debug=False,
                   num_devices=NCORES)
    # host pre-laid-out inputs: partition p holds gaussian j*128+p at free j
    pos = nc.dram_tensor("positions", [128, NCHUNK, 3], f32, kind="ExternalInput")
    col = nc.dram_tensor("colors", [128, NCHUNK, 3], bf16, kind="ExternalInput")
    opa = nc.dram_tensor("opacities", [128, NCHUNK], f32, kind="ExternalInput")
    sca = nc.dram_tensor("scales", [128, NCHUNK], f32, kind="ExternalInput")
    rt = nc.dram_tensor("rt", [NPOSE, 12], f32, kind="ExternalInput")
    out = nc.dram_tensor("out", [RSROWS, 3, W], f32, kind="ExternalOutput")

    cxy = _const_blocks()
    constxy_d = nc.inline_tensor(np.asarray(cxy), name="constXY")
    ident_d = nc.inline_tensor(np.eye(128, dtype=ml_dtypes.bfloat16), name="ident")

    cc_in = nc.dram_tensor("cc_in", [NPOSE, 128, 512], f32, kind="Internal")
    cc_out = nc.dram_tensor("cc_out", [RSROWS, 512], f32, kind="Internal")

    groups = [list(range(NCORES))]

    with tile.TileContext(nc) as tc:
        with (
            tc.tile_pool(name="const", bufs=1) as cpool,
            tc.tile_pool(name="prep", bufs=2) as prep,
            tc.tile_pool(name="pk", bufs=2) as pkpool,
            tc.tile_pool(name="t32", bufs=3) as t32pool,
            tc.tile_pool(name="rhs4", bufs=3) as rhspool,
            tc.tile_pool(name="ey4", bufs=3) as eypool,
            tc.tile_pool(name="fin", bufs=1) as fin,
            tc.tile_pool(name="ps_tr", bufs=2, space="PSUM") as ps_tr,
            tc.tile_pool(name="ps_arg", bufs=2, space="PSUM") as ps_arg,
            tc.tile_pool(name="ps_acc", bufs=2, space="PSUM") as ps_acc,
        ):
            # ---- inputs to SBUF; order matters: the prep chain needs
            # scales/opacities/rt first, the first transpose needs positions
            # + ident, colors and the arg-MM constant come later ----
            scat = cpool.tile([128, NCHUNK], f32)
            nc.sync.dma_start(scat[:], sca.ap())
            opat = cpool.tile([128, NCHUNK], f32)
            nc.sync.dma_start(opat[:], opa.ap())
            rtb = cpool.tile([128, NPOSE * 12], f32)
            nc.sync.dma_start(rtb[:], rt.ap().rearrange("a b -> (a b)")
                              .partition_broadcast(128))
            pos_t = cpool.tile([128, NCHUNK, 3], f32)
            nc.sync.dma_start(pos_t[:], pos.ap())
            ident = cpool.tile([128, 128], bf16)
            nc.sync.dma_start(ident[:], ident_d.ap())
            constxy = cpool.tile([96, 1024], bf16)
            nc.sync.dma_start(constxy[:], constxy_d.ap())
            col3 = cpool.tile([128, NCHUNK, 3], bf16)
            nc.sync.dma_start(col3[:], col.ap())

            # ---- pose-independent per-gaussian prep ----
            opc = prep.tile([128, NCHUNK], f32, tag="opc")
            nc.vector.tensor_scalar_max(opc[:], opat[:], 1e-30)
            lnop = prep.tile([128, NCHUNK], f32, tag="lnop")
            nc.scalar.activation(lnop[:], opc[:], ACTF.Ln)

            s2 = prep.tile([128, NCHUNK], f32, tag="s2")
            nc.vector.tensor_tensor(s2[:], scat[:], scat[:], ALU.mult)
            m2s2 = prep.tile([128, NCHUNK], f32, tag="m2s2")
            nc.vector.tensor_scalar_mul(m2s2[:], s2[:], -2.0)
            niv = prep.tile([128, NCHUNK], f32, tag="niv")
            nc.vector.reciprocal(niv[:], m2s2[:])

            nivh = prep.tile([128, NCHUNK], bf16, tag="nivh")
            nc.vector.tensor_copy(nivh[:], niv[:])
            r1 = prep.tile([128, NCHUNK], f32, tag="r1")
            nc.vector.tensor_tensor(r1[:], niv[:], nivh[:], ALU.subtract)
            nivm = prep.tile([128, NCHUNK], bf16, tag="nivm")
            nc.vector.tensor_copy(nivm[:], r1[:])
            r2 = prep.tile([128, NCHUNK], f32, tag="r2")
            nc.vector.tensor_tensor(r2[:], r1[:], nivm[:], ALU.subtract)
            nivl = prep.tile([128, NCHUNK], bf16, tag="nivl")
            nc.vector.tensor_copy(nivl[:], r2[:])

            def rsc(p, k):
                return rtb[:, p * 12 + k: p * 12 + k + 1]

            def nivv(t):
                return (t[:].rearrange("p (b a) -> p b a", b=NBLK)
                        .unsqueeze(2))

            def build_packed(p):
                """Per-pose packed coefficient tile, block-major
                (bf16, [128, NBLK, NQ, 4]): slice [:, bb] is a contiguous
                [128, 96] transpose input."""
                packed = pkpool.tile([128, NBLK, NQ, 4], bf16, tag="pk",
                                     name=f"packed{p}")
                # quadratic niv slots (broadcast copies)
                nc.vector.tensor_copy(
                    packed[:, :, 12:15, :],
                    nivv(nivh).broadcast_to([128, NBLK, 3, 4]))
                nc.vector.tensor_copy(
                    packed[:, :, 15:17, :],
                    nivv(nivm).broadcast_to([128, NBLK, 2, 4]))
                nc.vector.tensor_copy(
                    packed[:, :, 17:18, :],
                    nivv(nivl).broadcast_to([128, NBLK, 1, 4]))
                nc.vector.tensor_copy(
                    packed[:, :, 18:21, :],
                    nivv(nivh).broadcast_to([128, NBLK, 3, 4]))
                nc.vector.tensor_copy(
                    packed[:, :, 21:23, :],
                    nivv(nivm).broadcast_to([128, NBLK, 2, 4]))
                nc.vector.tensor_copy(
                    packed[:, :, 23:24, :],
                    nivv(nivl).broadcast_to([128, NBLK, 1, 4]))

                # cam rows via fused (pos_k * R) + acc chains
                cam = prep.tile([128, 3, NCHUNK], f32, tag="cam")
                for crow in range(3):
                    cr = cam[:, crow, :]
                    nc.vector.tensor_scalar(cr, pos_t[:, :, 0],
                                            rsc(p, 3 * crow + 0),
                                            rsc(p, 9 + crow), ALU.mult, ALU.add)
                    for k in (1, 2):
                        nc.vector.scalar_tensor_tensor(
                            cr, pos_t[:, :, k], rsc(p, 3 * crow + k), cr,
                            ALU.mult, ALU.add)

                zr = prep.tile([128, NCHUNK], f32, tag="zr")
                nc.vector.reciprocal(zr[:], cam[:, 2, :])

                # both axes at once: [128, 2, NCHUNK]
                zrb = zr[:].unsqueeze(1).broadcast_to([128, 2, NCHUNK])
                nivb = niv[:].unsqueeze(1).broadcast_to([128, 2, NCHUNK])
                t2 = prep.tile([128, 2, NCHUNK], f32, tag="t2")
                nc.vector.tensor_tensor(t2[:], cam[:, 0:2, :], zrb, ALU.mult)
                u2 = prep.tile([128, 2, NCHUNK], f32, tag="u2")
                nc.vector.tensor_scalar(u2[:], t2[:], float(FX),
                                        float(CX - CENT), ALU.mult, ALU.add)
                ucl = prep.tile([128, 2, NCHUNK], f32, tag="ucl")
                nc.vector.tensor_scalar(ucl[:], u2[:], -UCLAMP, UCLAMP,
                                        ALU.max, ALU.min)
                w2 = prep.tile([128, 2, NCHUNK], f32, tag="w2")
                nc.vector.tensor_tensor(w2[:], ucl[:], ucl[:], ALU.mult)

                # cf layout [a_x, a_y, b_x, b_y] matching q-slots 0..3
                cf = prep.tile([128, 4, NCHUNK], f32, tag="cf")
                nc.vector.tensor_tensor(cf[:, 0:2, :], w2[:], nivb, ALU.mult)
                nc.vector.tensor_tensor(cf[:, 0, :], cf[:, 0, :], lnop[:],
                                        ALU.add)
                nc.vector.scalar_tensor_tensor(cf[:, 2:4, :], ucl[:], -2.0,
                                               nivb, ALU.mult, ALU.mult)

                cfv = cf[:].rearrange("p c (b a) -> p b c a", b=NBLK)
                nc.vector.tensor_copy(packed[:, :, 0:4, :], cfv)
                sr1 = prep.tile([128, NBLK, 4, 4], f32, tag="sr1")
                nc.vector.tensor_tensor(sr1[:], cfv, packed[:, :, 0:4, :],
                                        ALU.subtract)
                nc.vector.tensor_copy(packed[:, :, 4:8, :], sr1[:])
                sr2 = prep.tile([128, NBLK, 4, 4], f32, tag="sr2")
                nc.vector.tensor_tensor(sr2[:], sr1[:], packed[:, :, 4:8, :],
                                        ALU.subtract)
                nc.vector.tensor_copy(packed[:, :, 8:12, :], sr2[:])
                return packed

            def phase_a(p, packed, bb):
                """Transpose + arg MMs + exp + colors for 4-chunk block bb.
                Returns (ey4, rhs4) bf16 tiles."""
                ptr = ps_tr.tile([128, 128], bf16, tag="tr")
                nc.tensor.transpose(
                    ptr[0:96, :],
                    packed[:, bb].rearrange("p q a -> p (q a)"),
                    ident[:])
                t32 = t32pool.tile([128, 128], bf16, tag="t32")
                nc.vector.tensor_copy(t32[0:96, :], ptr[0:96, :])

                parg = ps_arg.tile([128, 1024], f32, tag="arg")
                nc.tensor.matmul(parg[:, 0:512], t32[0:96, :],
                                 constxy[:, 0:512], start=True, stop=True)
                nc.tensor.matmul(parg[:, 512:1024], t32[0:96, :],
                                 constxy[:, 512:1024], start=True, stop=True)
                rhs4 = rhspool.tile([128, 4, 512], bf16, tag="rhs4")
                ey4 = eypool.tile([128, 4, 128], bf16, tag="ey4")
                nc.scalar.activation(rhs4[:, :, 384:512],
                                     parg[:, 0:512]
                                     .rearrange("p (a x) -> p a x", a=4),
                                     ACTF.Exp)
                nc.scalar.activation(ey4[:, :, :],
                                     parg[:, 512:1024]
                                     .rearrange("p (a x) -> p a x", a=4),
                                     ACTF.Exp)
                eng = nc.gpsimd if bb % 3 == 2 else nc.vector
                eng.tensor_tensor(
                    rhs4[:, :, 0:384].rearrange("p a (c x) -> p a c x", c=3),
                    rhs4[:, :, 384:512].unsqueeze(2)
                    .broadcast_to([128, 4, 3, 128]),
                    col3[:, 4 * bb: 4 * bb + 4, :].unsqueeze(3)
                    .broadcast_to([128, 4, 3, 128]),
                    ALU.mult)
                return ey4, rhs4

            packed_all = [None, None]
            for p in range(NPOSE):
                if p == 0:
                    packed_all[0] = build_packed(0)
                packed = packed_all[p]
                pacc = ps_acc.tile([128, 512], f32, tag="acc")
                pending = None
                for bb in range(NBLK):
                    tiles = phase_a(p, packed, bb)
                    if p == 0 and bb == 0:
                        # pose-1 prep fills the DVE while the PE chews on
                        # pose-0 blocks
                        packed_all[1] = build_packed(1)
                    if pending is not None:
                        ey4, rhs4 = pending
                        for k in range(4):
                            nc.tensor.matmul(
                                pacc[:], ey4[:, k, :], rhs4[:, k, :],
                                start=(bb == 1 and k == 0), stop=False)
                    pending = tiles
                ey4, rhs4 = pending
                for k in range(4):
                    nc.tensor.matmul(pacc[:], ey4[:, k, :], rhs4[:, k, :],
                                     start=False, stop=(k == 3))

                acc_sb = fin.tile([128, 512], f32, tag=f"accsb{p}")
                nc.scalar.copy(acc_sb[:], pacc[:])
                nc.sync.dma_start(cc_in.ap()[p], acc_sb[:])

            # ---- single collective: both poses' partials at once ----
            nc.gpsimd.collective_compute(
                "ReduceScatter", ALU.add, replica_groups=groups,
                ins=[cc_in.ap().rearrange("p r c -> (p r) c").opt()],
                outs=[cc_out.ap().opt()])
            shard = fin.tile([RSROWS, 512], f32, tag="shard")
            nc.sync.dma_start(shard[:], cc_out.ap())

            # ---- final: divide own 32-row shard, write it out ----
            dplus = fin.tile([RSROWS, 128], f32, tag="dplus")
            nc.vector.tensor_scalar_add(dplus[:], shard[:, 384:512],
                                        float(eps_total))
            rcp = fin.tile([RSROWS, 128], f32, tag="rcp")
            nc.vector.reciprocal(rcp[:], dplus[:])
            img = fin.tile([RSROWS, 3, 128], f32, tag="img")
            nc.vector.tensor_tensor(
                img[:], shard[:, 0:384].rearrange("p (c x) -> p c x", c=3),
                rcp[:].unsqueeze(1).broadcast_to([RSROWS, 3, 128]),
                ALU.mult)
            nc.sync.dma_start(out.ap(), img[:])

    nc.compile()
    return nc


def _install_ntff_hook():
    """Best-effort: register the axon NTFF profiling hook so trace=True
    yields exec_time_ns + a perfetto trace. The image's antenv package
    lacks axon_hooks, so bass_utils would otherwise skip tracing."""
    try:
        import sys
        import types
        mod = sys.modules.get("antenv.axon_hooks")
        if mod is None:
            import antenv
            mod = types.ModuleType("antenv.axon_hooks")
            mod._hook = None
            mod.set_axon_ntff_profile_hook = lambda h: setattr(mod, "_hook", h)
            mod.get_axon_ntff_profile_hook = lambda: mod._hook
            sys.modules["antenv.axon_hooks"] = mod
            antenv.axon_hooks = mod
        if mod.get_axon_ntff_profile_hook() is None:
            from trn_agent_boot.trn_boot import _ntff_profile_via_ctypes
            hook = _ntff_profile_via_ctypes("/opt/axon/libaxon_pjrt.so")
            if hook is None:
                return False
            mod.set_axon_ntff_profile_hook(hook)
        return True
    except Exception:
        return False


_CACHE = {}


def _get_nc(eps_total: float):
    key = float(eps_total)
    if key not in _CACHE:
        _CACHE[key] = _build(key)
    return _CACHE[key]


def kernel(positions, colors, opacities, scales, qvec, tvec,
           tile_hw=32, chunk_gauss=4096):
    positions = np.asarray(positions, np.float32)
    colors = np.asarray(colors, np.float32)
    opacities = np.asarray(opacities, np.float32)
    scales = np.asarray(scales, np.float32)
    qvec = np.asarray(qvec, np.float32)
    tvec = np.asarray(tvec, np.float32)
    tile_hw = int(tile_hw)
    chunk_gauss = int(chunk_gauss)
    n = positions.shape[0]
    assert n == NG and tile_hw == 32, (n, tile_hw)
    eps_total = (n // chunk_gauss) * 1e-8

    rtv = np.zeros((NPOSE, 12), np.float32)
    for p in range(NPOSE):
        rtv[p, :9] = _quat2mat(qvec[p]).astype(np.float32).reshape(9)
        rtv[p, 9:12] = tvec[p]

    def lay(a, shape, dt=np.float32):
        return np.ascontiguousarray(
            a.reshape(NCHUNK, 128, -1).transpose(1, 0, 2)
            .reshape(shape).astype(dt))

    in_maps = []
    for c in range(NCORES):
        sl = slice(c * NGC, (c + 1) * NGC)
        in_maps.append({
            "positions": lay(positions[sl], (128, NCHUNK, 3)),
            "colors": lay(colors[sl], (128, NCHUNK, 3), ml_dtypes.bfloat16),
            "opacities": lay(opacities[sl], (128, NCHUNK)),
            "scales": lay(scales[sl], (128, NCHUNK)),
            "rt": rtv,
        })

    nc = _get_nc(eps_total)
    res = None
    trace = os.environ.get("KERNEL_TRACE", "1") == "1"
    if trace:
        trace = _install_ntff_hook()
    for attempt in range(3):
        try:
            res = run_bass_kernel_spmd(nc, in_maps, core_ids=list(range(NCORES)),
                                       trace=trace)
            break
        except Exception:
            if attempt == 2:
                raise
    global LAST_RESULT
    LAST_RESULT = res
    if res.exec_time_ns is not None:
        print(f"HW exec time: {res.exec_time_ns} ns")
    full = np.empty((NPOSE, 3, H, W), np.float32)
    rows_per_core = RSROWS
    for c in range(NCORES):
        sh = res.results[c]["out"]       # [RSROWS, 3, W]
        p, r0 = divmod(c * rows_per_core, H)
        full[p, :, r0:r0 + rows_per_core, :] = sh.transpose(1, 0, 2)
    return (full.reshape(NPOSE, 3, 16, 1024).transpose(0, 2, 1, 3)
            .reshape(NPOSE * 16, 3, tile_hw, tile_hw).astype(np.float32))


# revision 15
# speedup vs baseline: 1.1695x; 1.1225x over previous
"""Trainium2 Bass kernel for the differentiable Gaussian renderer.

Math: for each pose, each gaussian g splats w[g,p] = op_g * exp(-0.5*d2/var_g)
onto pixels p; output = (sum_g w*color) / (sum_g w + n_chunks*eps), tiled.

Key structure exploited: the Gaussian is separable, exp(-(dx^2+dy^2)*s) =
Ex(c) * Ey(r), where dx depends only on the pixel column and dy only on the
row.  Per gaussian we need just 256 exp evaluations instead of 16384, and the
pixel accumulation becomes, per 128-gaussian chunk, one K=128 matmul:

  acc[r, ch*128+c] += sum_g Ey[g,r] * rhs[g, ch*128+c],
  rhs[g, ch*128+c] = Ex[g,c] * colors[g, ch]   (ch==3 column is Ex itself,
                                                giving the denominator)

The exp arguments arg_x[g,c] = niv_g*(c'-u'_g)^2 + ln(op_g) (and arg_y) are
produced on the tensor engine: per 4-chunk block, the per-gaussian bf16
coefficients (split 2-way hi/mid, 14 q-slots per chunk, block-major layout)
are PE-transposed into a [4*14, 128] layout, then two K=56 bf16 matmuls
against a block-diagonal constant matrix of {1, c', c'^2} rows produce the
block's x-args and y-args (one PSUM bank each); a single fused ACT exp
writes the denominator Ex and the stationary Ey side by side in bf16.
Opacity rides in ln-space inside arg_x; u,v are clamped to +-110.5 around
the image center (gaussians beyond that have w == 0 in fp32 anyway).
Residual-split error is dominated by the bf16 rounding of Ex/Ey either way
(~1.2e-3 rel, measured against an fp64 oracle in numpy).

Sharding: gaussians are split 8 ways (8192/core).  Each core accumulates
partial num/den for a pose in a single PSUM bank (64 chunk matmuls); a
per-pose ReduceScatter(add) combines partials across cores (pose 0's RS
overlaps pose 1's rendering; the CC bootstrap barrier ends ~57us into the
run regardless, right around when pose 0's partials are ready).  Core c
receives image rows 16c..16c+16 of each pose, divides only its shard after
BOTH scatters land (the joint dependency pins the divide at the true end of
the program so the tile scheduler cannot hoist it into the render stream),
and the host gathers the 8 shards into the two full images.

Pipeline: per 4-chunk block, phase A = PE transpose -> 2 arg-MMs -> fused
ACT exp (bf16) -> color scale (broadcast bf16 multiply, chunks 0-1 on DVE,
2-3 on GpSimd); the block's 4 bf16 main-MMs are emitted one block behind so
the PE always has phase-A work while colors finish.  Pose 1's coefficient
prep is emitted at deprioritized scheduler priority so it backfills DVE
idle slots during pose 0's render instead of delaying packed0.
"""
import os
import numpy as np
import ml_dtypes

import concourse.mybir as mybir
import concourse.tile as tile
import concourse.bacc as bacc
from concourse.bass_utils import run_bass_kernel_spmd

f32 = mybir.dt.float32
bf16 = mybir.dt.bfloat16
ALU = mybir.AluOpType
ACTF = mybir.ActivationFunctionType

NCORES = 8
NPOSE = 2
H = W = 128
FX = FY = 120.0
CX = CY = 64.0
NG = 65536
NGC = NG // NCORES          # gaussians per core
NCHUNK = NGC // 128         # 64 chunks of 128 gaussians
NBLK = NCHUNK // 4          # 16 transpose blocks of 4 chunks
CENT = 63.5
UCLAMP = 110.5
NQ = 14                     # q-slots per chunk
KQ = NQ * 4                 # transpose rows per block
RSROWS = H // NCORES        # image rows per core after each ReduceScatter


def _bf(x):
    return np.asarray(x).astype(ml_dtypes.bfloat16)


def _split2(x):
    h = _bf(x).astype(np.float64)
    m = _bf(x - h).astype(np.float64)
    return h, m


def _const_blocks():
    """[KQ, 1024] bf16 block-diagonal matmul constant.  Row q*4+a carries
    chunk a's columns only (packed stores a 4-chunk block as [q, a]
    interleaved rows).

    q-slot layout per chunk (matches build_packed writes):
      0-3   a_x, a_y, b_x, b_y  (hi)
      4-7   same (mid)
      8-10  x quadratic: nivh*c2h, nivh*c2m, nivm*c2h
      11-13 y quadratic, mirrored
    """
    cp = np.arange(128, dtype=np.float64) - CENT
    c2 = cp * cp
    c2h, c2m = _split2(c2)
    ones = np.ones(128)
    zer = np.zeros(128)
    xrows, yrows = [], []
    for _ in range(2):                       # h / m coefficient groups
        xrows += [ones, zer, cp, zer]
        yrows += [zer, ones, zer, cp]
    xrows += [c2h, c2m, c2h] + [zer] * 3
    yrows += [zer] * 3 + [c2h, c2m, c2h]
    bx, by = np.stack(xrows), np.stack(yrows)
    cxy = np.zeros((KQ, 1024))
    for q in range(NQ):
        for a in range(4):
            cxy[q * 4 + a, 128 * a:128 * a + 128] = bx[q]
            cxy[q * 4 + a, 512 + 128 * a:512 + 128 * a + 128] = by[q]
    return _bf(cxy)


def _quat2mat(q):
    q = np.asarray(q, np.float64)
    q = q / np.linalg.norm(q)
    w, x, y, z = q
    return np.array([
        [1 - 2 * (y * y + z * z), 2 * (x * y - z * w), 2 * (x * z + y * w)],
        [2 * (x * y + z * w), 1 - 2 * (x * x + z * z), 2 * (y * z - x * w)],
        [2 * (x * z - y * w), 2 * (y * z + x * w), 1 - 2 * (x * x + y * y)],
    ])


def _build(eps_total: float):
    nc = bacc.Bacc("TRN2", target_bir_lowering=False, debug=False,
                   num_devices=NCORES)
    # host pre-laid-out inputs: partition p holds gaussian j*128+p at free j
    pos = nc.dram_tensor("positions", [128, NCHUNK, 3], f32, kind="ExternalInput")
    col = nc.dram_tensor("colors", [128, NCHUNK, 3], bf16, kind="ExternalInput")
    opa = nc.dram_tensor("opacities", [128, NCHUNK], f32, kind="ExternalInput")
    sca = nc.dram_tensor("scales", [128, NCHUNK], f32, kind="ExternalInput")
    rt = nc.dram_tensor("rt", [NPOSE, 12], f32, kind="ExternalInput")
    out = nc.dram_tensor("out", [RSROWS, NPOSE, 3, W], f32,
                         kind="ExternalOutput")

    cxy = _const_blocks()
    constxy_d = nc.inline_tensor(np.asarray(cxy), name="constXY")
    ident_d = nc.inline_tensor(np.eye(128, dtype=ml_dtypes.bfloat16), name="ident")

    cc_in = nc.dram_tensor("cc_in", [NPOSE, 128, 512], f32, kind="Internal")
    cc_out = nc.dram_tensor("cc_out", [NPOSE, RSROWS, 512], f32, kind="Internal")

    groups = [list(range(NCORES))]

    with tile.TileContext(nc) as tc:
        with (
            tc.tile_pool(name="const", bufs=1) as cpool,
            tc.tile_pool(name="prep", bufs=2) as prep,
            tc.tile_pool(name="pk", bufs=2) as pkpool,
            tc.tile_pool(name="t32", bufs=3) as t32pool,
            tc.tile_pool(name="rhs4", bufs=3) as rhspool,
            tc.tile_pool(name="fin", bufs=1) as fin,
            tc.tile_pool(name="ps_tr", bufs=2, space="PSUM") as ps_tr,
            tc.tile_pool(name="ps_arg", bufs=2, space="PSUM") as ps_arg,
            tc.tile_pool(name="ps_acc", bufs=2, space="PSUM") as ps_acc,
        ):
            # ---- inputs to SBUF; order matters: the prep chain needs
            # scales/opacities/rt first, the first transpose needs positions
            # + ident, colors and the arg-MM constant come later ----
            scat = cpool.tile([128, NCHUNK], f32)
            nc.sync.dma_start(scat[:], sca.ap())
            opat = cpool.tile([128, NCHUNK], f32)
            nc.sync.dma_start(opat[:], opa.ap())
            rtb = cpool.tile([128, NPOSE * 12], f32)
            nc.sync.dma_start(rtb[:], rt.ap().rearrange("a b -> (a b)")
                              .partition_broadcast(128))
            pos_t = cpool.tile([128, NCHUNK, 3], f32)
            nc.sync.dma_start(pos_t[:], pos.ap())
            ident = cpool.tile([128, 128], bf16)
            nc.sync.dma_start(ident[:], ident_d.ap())
            constxy = cpool.tile([KQ, 1024], bf16)
            nc.sync.dma_start(constxy[:], constxy_d.ap())
            col3 = cpool.tile([128, NCHUNK, 3], bf16)
            nc.sync.dma_start(col3[:], col.ap())

            # ---- pose-independent per-gaussian prep ----
            opc = prep.tile([128, NCHUNK], f32, tag="opc")
            nc.vector.tensor_scalar_max(opc[:], opat[:], 1e-30)
            lnop = prep.tile([128, NCHUNK], f32, tag="lnop")
            nc.scalar.activation(lnop[:], opc[:], ACTF.Ln)

            s2 = prep.tile([128, NCHUNK], f32, tag="s2")
            nc.vector.tensor_tensor(s2[:], scat[:], scat[:], ALU.mult)
            m2s2 = prep.tile([128, NCHUNK], f32, tag="m2s2")
            nc.vector.tensor_scalar_mul(m2s2[:], s2[:], -2.0)
            niv = prep.tile([128, NCHUNK], f32, tag="niv")
            nc.vector.reciprocal(niv[:], m2s2[:])

            nivh = prep.tile([128, NCHUNK], bf16, tag="nivh")
            nc.vector.tensor_copy(nivh[:], niv[:])
            r1 = prep.tile([128, NCHUNK], f32, tag="r1")
            nc.vector.tensor_tensor(r1[:], niv[:], nivh[:], ALU.subtract)
            nivm = prep.tile([128, NCHUNK], bf16, tag="nivm")
            nc.vector.tensor_copy(nivm[:], r1[:])

            def rsc(p, k):
                return rtb[:, p * 12 + k: p * 12 + k + 1]

            def nivv(t):
                return (t[:].rearrange("p (b a) -> p b a", b=NBLK)
                        .unsqueeze(2))

            def build_packed(p):
                """Per-pose packed coefficient tile, block-major
                (bf16, [128, NBLK, NQ, 4]): slice [:, bb] is a contiguous
                [128, KQ] transpose input."""
                packed = pkpool.tile([128, NBLK, NQ, 4], bf16, tag="pk",
                                     name=f"packed{p}")
                # quadratic niv slots (broadcast copies)
                nc.vector.tensor_copy(
                    packed[:, :, 8:10, :],
                    nivv(nivh).broadcast_to([128, NBLK, 2, 4]))
                nc.vector.tensor_copy(
                    packed[:, :, 10:11, :],
                    nivv(nivm).broadcast_to([128, NBLK, 1, 4]))
                nc.vector.tensor_copy(
                    packed[:, :, 11:13, :],
                    nivv(nivh).broadcast_to([128, NBLK, 2, 4]))
                nc.vector.tensor_copy(
                    packed[:, :, 13:14, :],
                    nivv(nivm).broadcast_to([128, NBLK, 1, 4]))

                # cam rows via fused (pos_k * R) + acc chains
                cam = prep.tile([128, 3, NCHUNK], f32, tag="cam")
                for crow in range(3):
                    cr = cam[:, crow, :]
                    nc.vector.tensor_scalar(cr, pos_t[:, :, 0],
                                            rsc(p, 3 * crow + 0),
                                            rsc(p, 9 + crow), ALU.mult, ALU.add)
                    for k in (1, 2):
                        nc.vector.scalar_tensor_tensor(
                            cr, pos_t[:, :, k], rsc(p, 3 * crow + k), cr,
                            ALU.mult, ALU.add)

                zr = prep.tile([128, NCHUNK], f32, tag="zr")
                nc.vector.reciprocal(zr[:], cam[:, 2, :])

                # both axes at once: [128, 2, NCHUNK]
                zrb = zr[:].unsqueeze(1).broadcast_to([128, 2, NCHUNK])
                nivb = niv[:].unsqueeze(1).broadcast_to([128, 2, NCHUNK])
                t2 = prep.tile([128, 2, NCHUNK], f32, tag="t2")
                nc.vector.tensor_tensor(t2[:], cam[:, 0:2, :], zrb, ALU.mult)
                u2 = prep.tile([128, 2, NCHUNK], f32, tag="u2")
                nc.vector.tensor_scalar(u2[:], t2[:], float(FX),
                                        float(CX - CENT), ALU.mult, ALU.add)
                ucl = prep.tile([128, 2, NCHUNK], f32, tag="ucl")
                nc.vector.tensor_scalar(ucl[:], u2[:], -UCLAMP, UCLAMP,
                                        ALU.max, ALU.min)
                w2 = prep.tile([128, 2, NCHUNK], f32, tag="w2")
                nc.vector.tensor_tensor(w2[:], ucl[:], ucl[:], ALU.mult)

                # cf layout [a_x, a_y, b_x, b_y] matching q-slots 0..3
                cf = prep.tile([128, 4, NCHUNK], f32, tag="cf")
                nc.vector.tensor_tensor(cf[:, 0:2, :], w2[:], nivb, ALU.mult)
                nc.vector.tensor_tensor(cf[:, 0, :], cf[:, 0, :], lnop[:],
                                        ALU.add)
                nc.vector.scalar_tensor_tensor(cf[:, 2:4, :], ucl[:], -2.0,
                                               nivb, ALU.mult, ALU.mult)

                cfv = cf[:].rearrange("p c (b a) -> p b c a", b=NBLK)
                nc.vector.tensor_copy(packed[:, :, 0:4, :], cfv)
                sr1 = prep.tile([128, NBLK, 4, 4], f32, tag="sr1")
                nc.vector.tensor_tensor(sr1[:], cfv, packed[:, :, 0:4, :],
                                        ALU.subtract)
                nc.vector.tensor_copy(packed[:, :, 4:8, :], sr1[:])
                return packed

            def phase_a(p, packed, bb):
                """Transpose + arg MMs + fused exp + colors for 4-chunk
                block bb.  Returns the rhs4 tile (colors 0:384, den Ex
                384:512, Ey 512:640)."""
                ptr = ps_tr.tile([128, 128], bf16, tag="tr")
                nc.tensor.transpose(
                    ptr[0:KQ, :],
                    packed[:, bb].rearrange("p q a -> p (q a)"),
                    ident[:])
                t32 = t32pool.tile([128, 128], bf16, tag="t32")
                nc.vector.tensor_copy(t32[0:KQ, :], ptr[0:KQ, :])

                parg = ps_arg.tile([128, 1024], f32, tag="arg")
                nc.tensor.matmul(parg[:, 0:512], t32[0:KQ, :],
                                 constxy[:, 0:512], start=True, stop=True)
                nc.tensor.matmul(parg[:, 512:1024], t32[0:KQ, :],
                                 constxy[:, 512:1024], start=True, stop=True)
                rhs4 = rhspool.tile([128, 4, 640], bf16, tag="rhs4")
                nc.scalar.activation(
                    rhs4[:, :, 384:640].rearrange("p a (two x) -> p two a x",
                                                  two=2),
                    parg[:].rearrange("p (two a x) -> p two a x",
                                      two=2, a=4),
                    ACTF.Exp)
                for eng, asl in ((nc.vector, slice(0, 2)),
                                 (nc.gpsimd, slice(2, 4))):
                    eng.tensor_tensor(
                        rhs4[:, asl, 0:384]
                        .rearrange("p a (c x) -> p a c x", c=3),
                        rhs4[:, asl, 384:512].unsqueeze(2)
                        .broadcast_to([128, 2, 3, 128]),
                        col3[:, 4 * bb + asl.start: 4 * bb + asl.stop, :]
                        .unsqueeze(3).broadcast_to([128, 2, 3, 128]),
                        ALU.mult)
                return rhs4

            packed_all = [None, None]
            packed_all[0] = build_packed(0)
            with tc.high_priority(offset=-10_000_000):
                # deprioritized: backfills DVE idle slots during pose 0
                packed_all[1] = build_packed(1)
            for p in range(NPOSE):
                packed = packed_all[p]
                pacc = ps_acc.tile([128, 512], f32, tag="acc")
                pending = None
                for bb in range(NBLK):
                    tiles = phase_a(p, packed, bb)
                    if pending is not None:
                        rhs4 = pending
                        for k in range(4):
                            nc.tensor.matmul(
                                pacc[:], rhs4[:, k, 512:640],
                                rhs4[:, k, 0:512],
                                start=(bb == 1 and k == 0), stop=False)
                    pending = tiles
                rhs4 = pending
                for k in range(4):
                    nc.tensor.matmul(pacc[:], rhs4[:, k, 512:640],
                                     rhs4[:, k, 0:512],
                                     start=False, stop=(k == 3))

                acc_sb = fin.tile([128, 512], f32, tag=f"accsb{p}")
                nc.scalar.copy(acc_sb[:], pacc[:])
                nc.sync.dma_start(cc_in.ap()[p], acc_sb[:])
                nc.gpsimd.collective_compute(
                    "ReduceScatter", ALU.add, replica_groups=groups,
                    ins=[cc_in.ap()[p].opt()],
                    outs=[cc_out.ap()[p].opt()])

            # ---- final: divide own shard of both poses, write it out ----
            shard = fin.tile([RSROWS, NPOSE, 512], f32, tag="shard")
            for p in range(NPOSE):
                nc.sync.dma_start(shard[:, p, :], cc_out.ap()[p])
            dplus = fin.tile([RSROWS, NPOSE, 128], f32, tag="dplus")
            nc.vector.tensor_scalar_add(dplus[:], shard[:, :, 384:512],
                                        float(eps_total))
            rcp = fin.tile([RSROWS, NPOSE, 128], f32, tag="rcp")
            nc.vector.reciprocal(rcp[:], dplus[:])
            img = fin.tile([RSROWS, NPOSE, 3, 128], f32, tag="img")
            nc.vector.tensor_tensor(
                img[:], shard[:, :, 0:384]
                .rearrange("p t (c x) -> p t c x", c=3),
                rcp[:].unsqueeze(2).broadcast_to([RSROWS, NPOSE, 3, 128]),
                ALU.mult)
            nc.sync.dma_start(out.ap(), img[:])

    nc.compile()
    return nc


def _install_ntff_hook():
    """Best-effort: register the axon NTFF profiling hook so trace=True
    yields exec_time_ns + a perfetto trace. The image's antenv package
    lacks axon_hooks, so bass_utils would otherwise skip tracing."""
    try:
        import sys
        import types
        mod = sys.modules.get("antenv.axon_hooks")
        if mod is None:
            import antenv
            mod = types.ModuleType("antenv.axon_hooks")
            mod._hook = None
            mod.set_axon_ntff_profile_hook = lambda h: setattr(mod, "_hook", h)
            mod.get_axon_ntff_profile_hook = lambda: mod._hook
            sys.modules["antenv.axon_hooks"] = mod
            antenv.axon_hooks = mod
        if mod.get_axon_ntff_profile_hook() is None:
            from trn_agent_boot.trn_boot import _ntff_profile_via_ctypes
            hook = _ntff_profile_via_ctypes("/opt/axon/libaxon_pjrt.so")
            if hook is None:
                return False
            mod.set_axon_ntff_profile_hook(hook)
        return True
    except Exception:
        return False


_CACHE = {}


def _get_nc(eps_total: float):
    key = float(eps_total)
    if key not in _CACHE:
        _CACHE[key] = _build(key)
    return _CACHE[key]


def kernel(positions, colors, opacities, scales, qvec, tvec,
           tile_hw=32, chunk_gauss=4096):
    positions = np.asarray(positions, np.float32)
    colors = np.asarray(colors, np.float32)
    opacities = np.asarray(opacities, np.float32)
    scales = np.asarray(scales, np.float32)
    qvec = np.asarray(qvec, np.float32)
    tvec = np.asarray(tvec, np.float32)
    tile_hw = int(tile_hw)
    chunk_gauss = int(chunk_gauss)
    n = positions.shape[0]
    assert n == NG and tile_hw == 32, (n, tile_hw)
    eps_total = (n // chunk_gauss) * 1e-8

    rtv = np.zeros((NPOSE, 12), np.float32)
    for p in range(NPOSE):
        rtv[p, :9] = _quat2mat(qvec[p]).astype(np.float32).reshape(9)
        rtv[p, 9:12] = tvec[p]

    def lay(a, shape, dt=np.float32):
        return np.ascontiguousarray(
            a.reshape(NCHUNK, 128, -1).transpose(1, 0, 2)
            .reshape(shape).astype(dt))

    in_maps = []
    for c in range(NCORES):
        sl = slice(c * NGC, (c + 1) * NGC)
        in_maps.append({
            "positions": lay(positions[sl], (128, NCHUNK, 3)),
            "colors": lay(colors[sl], (128, NCHUNK, 3), ml_dtypes.bfloat16),
            "opacities": lay(opacities[sl], (128, NCHUNK)),
            "scales": lay(scales[sl], (128, NCHUNK)),
            "rt": rtv,
        })

    nc = _get_nc(eps_total)
    res = None
    trace = os.environ.get("KERNEL_TRACE", "1") == "1"
    if trace:
        trace = _install_ntff_hook()
    for attempt in range(3):
        try:
            res = run_bass_kernel_spmd(nc, in_maps, core_ids=list(range(NCORES)),
                                       trace=trace)
            break
        except Exception:
            if attempt == 2:
                raise
    global LAST_RESULT
    LAST_RESULT = res
    if res.exec_time_ns is not None:
        print(f"HW exec time: {res.exec_time_ns} ns")
    full = np.empty((NPOSE, 3, H, W), np.float32)
    for c in range(NCORES):
        sh = res.results[c]["out"]       # [RSROWS, NPOSE, 3, W]
        full[:, :, RSROWS * c:RSROWS * (c + 1), :] = sh.transpose(1, 2, 0, 3)
    return (full.reshape(NPOSE, 3, 16, 1024).transpose(0, 2, 1, 3)
            .reshape(NPOSE * 16, 3, tile_hw, tile_hw).astype(np.float32))


# revision 25
# speedup vs baseline: 1.1916x; 1.0188x over previous
"""Trainium2 Bass kernel for the differentiable Gaussian renderer.

Math: for each pose, each gaussian g splats w[g,p] = op_g * exp(-0.5*d2/var_g)
onto pixels p; output = (sum_g w*color) / (sum_g w + n_chunks*eps), tiled.

Key structure exploited: the Gaussian is separable, exp(-(dx^2+dy^2)*s) =
Ex(c) * Ey(r), where dx depends only on the pixel column and dy only on the
row.  Per gaussian we need just 256 exp evaluations instead of 16384, and the
pixel accumulation becomes, per 128-gaussian chunk, one K=128 matmul:

  acc[r, ch*128+c] += sum_g Ey[g,r] * rhs[g, ch*128+c],
  rhs[g, ch*128+c] = Ex[g,c] * colors[g, ch]   (ch==3 column is Ex itself,
                                                giving the denominator)

The exp arguments arg_x[g,c] = niv_g*(c'-u'_g)^2 + ln(op_g) (and arg_y) are
produced on the tensor engine: per 4-chunk block, the per-gaussian bf16
coefficients (split 2-way hi/mid, 14 q-slots per chunk, block-major layout)
are PE-transposed into a [4*14, 128] layout, then two K=56 bf16 matmuls
against a block-diagonal constant matrix of {1, c', c'^2} rows produce the
block's x-args and y-args (one PSUM bank each); a single fused ACT exp
writes the denominator Ex and the stationary Ey side by side in bf16.
Opacity rides in ln-space inside arg_x; u,v are clamped to +-110.5 around
the image center (gaussians beyond that have w == 0 in fp32 anyway).
Residual-split error is dominated by the bf16 rounding of Ex/Ey either way
(~1.2e-3 rel, measured against an fp64 oracle in numpy).

Sharding: gaussians are split 8 ways (8192/core).  Each core accumulates
partial num/den for a pose in a single PSUM bank (64 chunk matmuls); a
per-pose ReduceScatter(add) combines partials across cores (pose 0's RS
overlaps pose 1's rendering; the CC bootstrap barrier ends ~57us into the
run regardless, right around when pose 0's partials are ready).  Core c
receives image rows 16c..16c+16 of each pose, divides only its shard after
BOTH scatters land (the joint dependency pins the divide at the true end of
the program so the tile scheduler cannot hoist it into the render stream),
and the host gathers the 8 shards into the two full images.

Pipeline: per 4-chunk block, phase A = PE transpose -> 2 arg-MMs -> fused
ACT exp (bf16) -> color scale (broadcast bf16 multiply, chunks 0-1 on DVE,
2-3 on GpSimd); the block's 4 bf16 main-MMs are emitted one block behind so
the PE always has phase-A work while colors finish.  Pose 1's coefficient
prep is emitted at deprioritized scheduler priority so it backfills DVE
idle slots during pose 0's render instead of delaying packed0.
"""
import os
import numpy as np
import ml_dtypes

import concourse.mybir as mybir
import concourse.tile as tile
import concourse.bacc as bacc
from concourse.bass_utils import run_bass_kernel_spmd

f32 = mybir.dt.float32
bf16 = mybir.dt.bfloat16
ALU = mybir.AluOpType
ACTF = mybir.ActivationFunctionType

NCORES = 8
NPOSE = 2
H = W = 128
FX = FY = 120.0
CX = CY = 64.0
NG = 65536
NGC = NG // NCORES          # gaussians per core
NCHUNK = NGC // 128         # 64 chunks of 128 gaussians
NBLK = NCHUNK // 4          # 16 transpose blocks of 4 chunks
CENT = 63.5
UCLAMP = 110.5
NQ = 14                     # q-slots per chunk
KQ = NQ * 4                 # transpose rows per block
RSROWS = H // NCORES        # image rows per core after each ReduceScatter


def _bf(x):
    return np.asarray(x).astype(ml_dtypes.bfloat16)


def _split2(x):
    h = _bf(x).astype(np.float64)
    m = _bf(x - h).astype(np.float64)
    return h, m


def _const_blocks():
    """[KQ, 1024] bf16 block-diagonal matmul constant.  Row q*4+a carries
    chunk a's columns only (packed stores a 4-chunk block as [q, a]
    interleaved rows).

    q-slot layout per chunk (matches build_packed writes):
      0-3   a_x, a_y, b_x, b_y  (hi)
      4-7   same (mid)
      8-10  x quadratic: nivh*c2h, nivh*c2m, nivm*c2h
      11-13 y quadratic, mirrored
    """
    cp = np.arange(128, dtype=np.float64) - CENT
    c2 = cp * cp
    c2h, c2m = _split2(c2)
    ones = np.ones(128)
    zer = np.zeros(128)
    xrows, yrows = [], []
    for _ in range(2):                       # h / m coefficient groups
        xrows += [ones, zer, cp, zer]
        yrows += [zer, ones, zer, cp]
    xrows += [c2h, c2m, c2h] + [zer] * 3
    yrows += [zer] * 3 + [c2h, c2m, c2h]
    bx, by = np.stack(xrows), np.stack(yrows)
    cxy = np.zeros((KQ, 1024))
    for q in range(NQ):
        for a in range(4):
            cxy[q * 4 + a, 128 * a:128 * a + 128] = bx[q]
            cxy[q * 4 + a, 512 + 128 * a:512 + 128 * a + 128] = by[q]
    return _bf(cxy)


def _quat2mat(q):
    q = np.asarray(q, np.float64)
    q = q / np.linalg.norm(q)
    w, x, y, z = q
    return np.array([
        [1 - 2 * (y * y + z * z), 2 * (x * y - z * w), 2 * (x * z + y * w)],
        [2 * (x * y + z * w), 1 - 2 * (x * x + z * z), 2 * (y * z - x * w)],
        [2 * (x * z - y * w), 2 * (y * z + x * w), 1 - 2 * (x * x + y * y)],
    ])


def _build(eps_total: float):
    nc = bacc.Bacc("TRN2", target_bir_lowering=False, debug=False,
                   num_devices=NCORES)
    # host pre-laid-out inputs: partition p holds gaussian j*128+p at free j
    pos = nc.dram_tensor("positions", [128, NCHUNK, 3], f32, kind="ExternalInput")
    col = nc.dram_tensor("colors", [128, NCHUNK, 3], bf16, kind="ExternalInput")
    opa = nc.dram_tensor("opacities", [128, NCHUNK], f32, kind="ExternalInput")
    sca = nc.dram_tensor("scales", [128, NCHUNK], f32, kind="ExternalInput")
    rt = nc.dram_tensor("rt", [NPOSE, 12], f32, kind="ExternalInput")
    out = nc.dram_tensor("out", [RSROWS, NPOSE, 3, W], f32,
                         kind="ExternalOutput")

    cxy = _const_blocks()
    constxy_d = nc.inline_tensor(np.asarray(cxy), name="constXY")
    ident_d = nc.inline_tensor(np.eye(128, dtype=ml_dtypes.bfloat16), name="ident")

    cc_in = nc.dram_tensor("cc_in", [NPOSE, 128, 512], bf16, kind="Internal")
    cc_out = nc.dram_tensor("cc_out", [NPOSE, RSROWS, 512], bf16, kind="Internal")

    groups = [list(range(NCORES))]

    with tile.TileContext(nc) as tc:
        with (
            tc.tile_pool(name="const", bufs=1) as cpool,
            tc.tile_pool(name="prep", bufs=2) as prep,
            tc.tile_pool(name="pk", bufs=2) as pkpool,
            tc.tile_pool(name="t32", bufs=3) as t32pool,
            tc.tile_pool(name="rhs4", bufs=3) as rhspool,
            tc.tile_pool(name="fin", bufs=1) as fin,
            tc.tile_pool(name="ps_tr", bufs=2, space="PSUM") as ps_tr,
            tc.tile_pool(name="ps_arg", bufs=2, space="PSUM") as ps_arg,
            tc.tile_pool(name="ps_acc", bufs=2, space="PSUM") as ps_acc,
        ):
            # ---- inputs to SBUF; order matters: the prep chain needs
            # scales/opacities/rt first, the first transpose needs positions
            # + ident, colors and the arg-MM constant come later ----
            scat = cpool.tile([128, NCHUNK], f32)
            nc.sync.dma_start(scat[:], sca.ap())
            opat = cpool.tile([128, NCHUNK], f32)
            nc.sync.dma_start(opat[:], opa.ap())
            rtb = cpool.tile([128, NPOSE * 12], f32)
            nc.sync.dma_start(rtb[:], rt.ap().rearrange("a b -> (a b)")
                              .partition_broadcast(128))
            pos_t = cpool.tile([128, NCHUNK, 3], f32)
            nc.sync.dma_start(pos_t[:], pos.ap())
            ident = cpool.tile([128, 128], bf16)
            nc.sync.dma_start(ident[:], ident_d.ap())
            constxy = cpool.tile([KQ, 1024], bf16)
            nc.sync.dma_start(constxy[:], constxy_d.ap())
            col3 = cpool.tile([128, NCHUNK, 3], bf16)
            nc.sync.dma_start(col3[:], col.ap())

            # ---- pose-independent per-gaussian prep; the niv side-chain
            # runs on GpSimd (idle until colors start) so the DVE critical
            # path is just cam -> uv -> coefficients ----
            opc = prep.tile([128, NCHUNK], f32, tag="opc")
            nc.gpsimd.tensor_scalar_max(opc[:], opat[:], 1e-30)
            lnop = prep.tile([128, NCHUNK], f32, tag="lnop")
            nc.scalar.activation(lnop[:], opc[:], ACTF.Ln)

            s2 = prep.tile([128, NCHUNK], f32, tag="s2")
            nc.gpsimd.tensor_tensor(s2[:], scat[:], scat[:], ALU.mult)
            m2s2 = prep.tile([128, NCHUNK], f32, tag="m2s2")
            nc.gpsimd.tensor_scalar_mul(m2s2[:], s2[:], -2.0)
            niv = prep.tile([128, NCHUNK], f32, tag="niv")
            nc.vector.reciprocal(niv[:], m2s2[:])

            nivh = prep.tile([128, NCHUNK], bf16, tag="nivh")
            nc.gpsimd.tensor_copy(nivh[:], niv[:])
            r1 = prep.tile([128, NCHUNK], f32, tag="r1")
            nc.gpsimd.tensor_tensor(r1[:], niv[:], nivh[:], ALU.subtract)
            nivm = prep.tile([128, NCHUNK], bf16, tag="nivm")
            nc.gpsimd.tensor_copy(nivm[:], r1[:])

            def rsc(p, k):
                return rtb[:, p * 12 + k: p * 12 + k + 1]

            def nivv(t):
                return (t[:].rearrange("p (b a) -> p b a", b=NBLK)
                        .unsqueeze(2))

            def build_packed(p, eng):
                """Per-pose packed coefficient tile, block-major
                (bf16, [128, NBLK, NQ, 4]): slice [:, bb] is a contiguous
                [128, KQ] transpose input.  eng picks the engine for the
                elementwise chain (reciprocal always runs on DVE)."""
                packed = pkpool.tile([128, NBLK, NQ, 4], bf16, tag="pk",
                                     name=f"packed{p}")
                # quadratic niv slots (broadcast copies, off the DVE path)
                nc.gpsimd.tensor_copy(
                    packed[:, :, 8:10, :],
                    nivv(nivh).broadcast_to([128, NBLK, 2, 4]))
                nc.gpsimd.tensor_copy(
                    packed[:, :, 10:11, :],
                    nivv(nivm).broadcast_to([128, NBLK, 1, 4]))
                nc.gpsimd.tensor_copy(
                    packed[:, :, 11:13, :],
                    nivv(nivh).broadcast_to([128, NBLK, 2, 4]))
                nc.gpsimd.tensor_copy(
                    packed[:, :, 13:14, :],
                    nivv(nivm).broadcast_to([128, NBLK, 1, 4]))

                # cam rows via fused (pos_k * R) + acc chains; AP-scalar ops
                # (TensorScalarPtr) only exist on DVE
                cam = prep.tile([128, 3, NCHUNK], f32, tag="cam")
                for crow in range(3):
                    cr = cam[:, crow, :]
                    nc.vector.tensor_scalar(cr, pos_t[:, :, 0],
                                            rsc(p, 3 * crow + 0),
                                            rsc(p, 9 + crow), ALU.mult, ALU.add)
                    for k in (1, 2):
                        nc.vector.scalar_tensor_tensor(
                            cr, pos_t[:, :, k], rsc(p, 3 * crow + k), cr,
                            ALU.mult, ALU.add)

                zr = prep.tile([128, NCHUNK], f32, tag="zr")
                nc.vector.reciprocal(zr[:], cam[:, 2, :])

                # both axes at once: [128, 2, NCHUNK]
                zrb = zr[:].unsqueeze(1).broadcast_to([128, 2, NCHUNK])
                nivb = niv[:].unsqueeze(1).broadcast_to([128, 2, NCHUNK])
                t2 = prep.tile([128, 2, NCHUNK], f32, tag="t2")
                eng.tensor_tensor(t2[:], cam[:, 0:2, :], zrb, ALU.mult)
                u2 = prep.tile([128, 2, NCHUNK], f32, tag="u2")
                eng.tensor_scalar(u2[:], t2[:], float(FX),
                                  float(CX - CENT), ALU.mult, ALU.add)
                ucl = prep.tile([128, 2, NCHUNK], f32, tag="ucl")
                eng.tensor_scalar(ucl[:], u2[:], -UCLAMP, UCLAMP,
                                  ALU.max, ALU.min)
                w2 = prep.tile([128, 2, NCHUNK], f32, tag="w2")
                eng.tensor_tensor(w2[:], ucl[:], ucl[:], ALU.mult)

                # cf layout [a_x, a_y, b_x, b_y] matching q-slots 0..3
                cf = prep.tile([128, 4, NCHUNK], f32, tag="cf")
                eng.tensor_tensor(cf[:, 0:2, :], w2[:], nivb, ALU.mult)
                eng.tensor_tensor(cf[:, 0, :], cf[:, 0, :], lnop[:],
                                  ALU.add)
                if eng is nc.vector:
                    eng.scalar_tensor_tensor(cf[:, 2:4, :], ucl[:], -2.0,
                                             nivb, ALU.mult, ALU.mult)
                else:
                    b4 = prep.tile([128, 2, NCHUNK], f32, tag="b4")
                    eng.tensor_tensor(b4[:], ucl[:], nivb, ALU.mult)
                    eng.tensor_scalar_mul(cf[:, 2:4, :], b4[:], -2.0)

                cfv = cf[:].rearrange("p c (b a) -> p b c a", b=NBLK)
                eng.tensor_copy(packed[:, :, 0:4, :], cfv)
                sr1 = prep.tile([128, NBLK, 4, 4], f32, tag="sr1")
                eng.tensor_tensor(sr1[:], cfv, packed[:, :, 0:4, :],
                                  ALU.subtract)
                eng.tensor_copy(packed[:, :, 4:8, :], sr1[:])
                return packed

            def phase_a(p, packed, bb):
                """Transpose + arg MMs + fused exp + colors for 4-chunk
                block bb.  Returns the rhs4 tile (colors 0:384, den Ex
                384:512, Ey 512:640)."""
                ptr = ps_tr.tile([128, 128], bf16, tag="tr")
                nc.tensor.transpose(
                    ptr[0:KQ, :],
                    packed[:, bb].rearrange("p q a -> p (q a)"),
                    ident[:])
                t32 = t32pool.tile([128, 128], bf16, tag="t32")
                nc.scalar.copy(t32[0:KQ, :], ptr[0:KQ, :])

                parg = ps_arg.tile([128, 1024], f32, tag="arg")
                nc.tensor.matmul(parg[:, 0:512], t32[0:KQ, :],
                                 constxy[:, 0:512], start=True, stop=True)
                nc.tensor.matmul(parg[:, 512:1024], t32[0:KQ, :],
                                 constxy[:, 512:1024], start=True, stop=True)
                rhs4 = rhspool.tile([128, 4, 640], bf16, tag="rhs4")
                nc.scalar.activation(
                    rhs4[:, :, 384:640].rearrange("p a (two x) -> p two a x",
                                                  two=2),
                    parg[:].rearrange("p (two a x) -> p two a x",
                                      two=2, a=4),
                    ACTF.Exp)
                for eng, asl in ((nc.vector, slice(0, 3)),
                                 (nc.gpsimd, slice(3, 4))):
                    na = asl.stop - asl.start
                    eng.tensor_tensor(
                        rhs4[:, asl, 0:384]
                        .rearrange("p a (c x) -> p a c x", c=3),
                        rhs4[:, asl, 384:512].unsqueeze(2)
                        .broadcast_to([128, na, 3, 128]),
                        col3[:, 4 * bb + asl.start: 4 * bb + asl.stop, :]
                        .unsqueeze(3).broadcast_to([128, na, 3, 128]),
                        ALU.mult)
                return rhs4

            packed_all = [None, None]
            packed_all[0] = build_packed(0, nc.vector)
            with tc.high_priority(offset=-10_000_000):
                # deprioritized + on GpSimd: fills GpSimd idle slots during
                # pose 0 without touching the DVE prep critical path
                packed_all[1] = build_packed(1, nc.gpsimd)
            for p in range(NPOSE):
                packed = packed_all[p]
                pacc = ps_acc.tile([128, 512], f32, tag="acc")
                pending = None
                for bb in range(NBLK):
                    tiles = phase_a(p, packed, bb)
                    if pending is not None:
                        rhs4 = pending
                        for k in range(4):
                            nc.tensor.matmul(
                                pacc[:], rhs4[:, k, 512:640],
                                rhs4[:, k, 0:512],
                                start=(bb == 1 and k == 0), stop=False)
                    pending = tiles
                rhs4 = pending
                for k in range(4):
                    nc.tensor.matmul(pacc[:], rhs4[:, k, 512:640],
                                     rhs4[:, k, 0:512],
                                     start=False, stop=(k == 3))

                acc_sb = fin.tile([128, 512], bf16, tag=f"accsb{p}")
                nc.scalar.copy(acc_sb[:], pacc[:])
                nc.sync.dma_start(cc_in.ap()[p], acc_sb[:])
                nc.gpsimd.collective_compute(
                    "ReduceScatter", ALU.add, replica_groups=groups,
                    ins=[cc_in.ap()[p].opt()],
                    outs=[cc_out.ap()[p].opt()])

            # ---- final: divide own shard of both poses, write it out ----
            shard = fin.tile([RSROWS, NPOSE, 512], bf16, tag="shard")
            for p in range(NPOSE):
                nc.sync.dma_start(shard[:, p, :], cc_out.ap()[p])
            dplus = fin.tile([RSROWS, NPOSE, 128], f32, tag="dplus")
            nc.vector.tensor_scalar_add(dplus[:], shard[:, :, 384:512],
                                        float(eps_total))
            rcp = fin.tile([RSROWS, NPOSE, 128], f32, tag="rcp")
            nc.vector.reciprocal(rcp[:], dplus[:])
            img = fin.tile([RSROWS, NPOSE, 3, 128], f32, tag="img")
            nc.vector.tensor_tensor(
                img[:], shard[:, :, 0:384]
                .rearrange("p t (c x) -> p t c x", c=3),
                rcp[:].unsqueeze(2).broadcast_to([RSROWS, NPOSE, 3, 128]),
                ALU.mult)
            nc.sync.dma_start(out.ap(), img[:])

    nc.compile()
    return nc


def _install_ntff_hook():
    """Best-effort: register the axon NTFF profiling hook so trace=True
    yields exec_time_ns + a perfetto trace. The image's antenv package
    lacks axon_hooks, so bass_utils would otherwise skip tracing."""
    try:
        import sys
        import types
        mod = sys.modules.get("antenv.axon_hooks")
        if mod is None:
            import antenv
            mod = types.ModuleType("antenv.axon_hooks")
            mod._hook = None
            mod.set_axon_ntff_profile_hook = lambda h: setattr(mod, "_hook", h)
            mod.get_axon_ntff_profile_hook = lambda: mod._hook
            sys.modules["antenv.axon_hooks"] = mod
            antenv.axon_hooks = mod
        if mod.get_axon_ntff_profile_hook() is None:
            from trn_agent_boot.trn_boot import _ntff_profile_via_ctypes
            hook = _ntff_profile_via_ctypes("/opt/axon/libaxon_pjrt.so")
            if hook is None:
                return False
            mod.set_axon_ntff_profile_hook(hook)
        return True
    except Exception:
        return False


_CACHE = {}


def _get_nc(eps_total: float):
    key = float(eps_total)
    if key not in _CACHE:
        _CACHE[key] = _build(key)
    return _CACHE[key]


def kernel(positions, colors, opacities, scales, qvec, tvec,
           tile_hw=32, chunk_gauss=4096):
    positions = np.asarray(positions, np.float32)
    colors = np.asarray(colors, np.float32)
    opacities = np.asarray(opacities, np.float32)
    scales = np.asarray(scales, np.float32)
    qvec = np.asarray(qvec, np.float32)
    tvec = np.asarray(tvec, np.float32)
    tile_hw = int(tile_hw)
    chunk_gauss = int(chunk_gauss)
    n = positions.shape[0]
    assert n == NG and tile_hw == 32, (n, tile_hw)
    eps_total = (n // chunk_gauss) * 1e-8

    rtv = np.zeros((NPOSE, 12), np.float32)
    for p in range(NPOSE):
        rtv[p, :9] = _quat2mat(qvec[p]).astype(np.float32).reshape(9)
        rtv[p, 9:12] = tvec[p]

    def lay(a, shape, dt=np.float32):
        return np.ascontiguousarray(
            a.reshape(NCHUNK, 128, -1).transpose(1, 0, 2)
            .reshape(shape).astype(dt))

    in_maps = []
    for c in range(NCORES):
        sl = slice(c * NGC, (c + 1) * NGC)
        in_maps.append({
            "positions": lay(positions[sl], (128, NCHUNK, 3)),
            "colors": lay(colors[sl], (128, NCHUNK, 3), ml_dtypes.bfloat16),
            "opacities": lay(opacities[sl], (128, NCHUNK)),
            "scales": lay(scales[sl], (128, NCHUNK)),
            "rt": rtv,
        })

    nc = _get_nc(eps_total)
    res = None
    trace = os.environ.get("KERNEL_TRACE", "1") == "1"
    if trace:
        trace = _install_ntff_hook()
    for attempt in range(3):
        try:
            res = run_bass_kernel_spmd(nc, in_maps, core_ids=list(range(NCORES)),
                                       trace=trace)
            break
        except Exception:
            if attempt == 2:
                raise
    global LAST_RESULT
    LAST_RESULT = res
    if res.exec_time_ns is not None:
        print(f"HW exec time: {res.exec_time_ns} ns")
    full = np.empty((NPOSE, 3, H, W), np.float32)
    for c in range(NCORES):
        sh = res.results[c]["out"]       # [RSROWS, NPOSE, 3, W]
        full[:, :, RSROWS * c:RSROWS * (c + 1), :] = sh.transpose(1, 2, 0, 3)
    return (full.reshape(NPOSE, 3, 16, 1024).transpose(0, 2, 1, 3)
            .reshape(NPOSE * 16, 3, tile_hw, tile_hw).astype(np.float32))
